# revision 13
# baseline (speedup 1.0000x reference)
"""Trainium2 Bass kernel for nn_DeTree (NODE-style oblivious decision ensemble).

Tree-sharded over 8 cores (64 trees/core), full batch per core.

Fast path (oblivious path_map), v2 — fully software-pipelined:
  Host folds softmax(feat_attention), the temperature scale (0.5*exp(-lt))
  and the softmax denominator into the fv weights `cws`, so the device
  pipeline is pure matmul/elementwise/act with no frontend exp:
    1. PE: fv(m,nh) = cws[:,m-block]^T @ x^T[, nh-half]   (f32r)
    2. DVE/GPSIMD: u = min(fv+b, 1); bins = max(u, EPS);
       omb = clamp(1-u, EPS, 1-EPS)  (pg tile, parity-swapped row layout)
    3. ACT: glog = ln(pg)
    4. PE: lo-sums S2 (16 combos/tree) + replicated hi-sums S1
       (3r x 4 combos/tree) via constant 0/1 selection matmuls.
    5. ACT: E2 = exp(S2), E1 = exp(S1)
    6. PE: M1[t,(r,hi)] = sum_lo resp[t,hi*16+lo,r] * E2[t,lo] (block-diag)
    7. DVE: P = M1 * E1
    8. PE: out[t*3+r] = sum_hi P, 4 groups accumulated per psum tile.
  All stages interleave per tree-group g (PE order: S(g), M1(g-1), OP(g-2))
  so PE/ACT/DVE/GPSIMD stay busy concurrently; input DMAs are spread
  across engine queues so the first fv matmul starts ~1us in.
Generic path (any path_map): 2-trees-per-matmul leaf log-sum (64 leaves),
exp, response block-diag accumulation (v1, unchanged).
"""
import numpy as np
from contextlib import ExitStack

import concourse.bass as bass
import concourse.bacc as bacc
import concourse.tile as tile
import concourse.mybir as mybir
from concourse.bass_utils import run_bass_kernel_spmd

F32 = mybir.dt.float32
F32R = mybir.dt.float32r
AF = mybir.ActivationFunctionType
ALU = mybir.AluOpType

B = 1024          # batch
F = 512           # in_features
T = 512           # num_trees
D = 6             # depth
R = 3             # response_dim
NLEAF = 64
NCORES = 8
T_C = T // NCORES          # 64 trees per core
TPG = 8                    # trees per gate-tile group
NG = T_C // TPG            # 8 groups per core
MROW = 64                  # padded rows per fv M-tile half (48 real + 16 pad)
NPAIR = T_C // 2           # generic path: 32 tree-pairs per core
PAIRS_PER_EG = 16
EPS = 2.0 ** -20
NH = 2                     # batch halves (1024 = 2 x 512)
BH = B // NH               # 512
NLO = 16                   # 2^4 lo-combos (depths 0..3)
NHI = 4                    # 2^2 hi-combos (depths 4..5)

_CACHE = {}


def _is_oblivious(path_map):
    pm = np.asarray(path_map).reshape(NLEAF, D)
    exp = np.array([[2 * j + ((l >> j) & 1) for j in range(D)]
                    for l in range(NLEAF)], dtype=pm.dtype)
    return bool(np.array_equal(pm, exp))


# ───────────────────────── fast (v2) constants ────────────────────────────
# pg row layout per group parity (within its [128, B] tile):
#   even g: bins rows 0..47,  omb rows 64..111  (src fv partitions 0..47)
#   odd  g: bins rows 64..111, omb rows 0..47   (src fv partitions 64..111)
# The bins write is partition-UNSHIFTED from fv and carries the per-row
# b-vector AP; the omb write derives from bins with constant scalars only.
# All partition starts are 0/64 (hardware requires starts in {0,32,64,96});
# rows 48..63 are never written (memset once per ring buffer to stay
# ln-safe), rows 112..127 are never read.

def _rowv2(parity, s, t_loc, d):
    """pg-tile row of gate (d, s) for local tree t_loc. s=0: bins, s=1: omb."""
    if parity == 0:
        base = 0 if s == 0 else 64
    else:
        base = 64 if s == 0 else 0
    return base + 6 * t_loc + d


def _build_sel2_v2(parity):
    """[128, 128] lo-sum selection: col = 16*t_loc + lo, depths 0..3."""
    S = np.zeros((128, 128), np.float32)
    for t_loc in range(TPG):
        for lo in range(NLO):
            col = NLO * t_loc + lo
            for j in range(4):
                S[_rowv2(parity, (lo >> j) & 1, t_loc, j), col] = 1.0
    return S


def _build_sel1_v2(parity):
    """[128, 96] replicated hi-sum selection: col = 12*t_loc + 4*r + hi."""
    S = np.zeros((128, 96), np.float32)
    for t_loc in range(TPG):
        for r in range(R):
            for hi in range(NHI):
                col = 12 * t_loc + 4 * r + hi
                for j in range(4, 6):
                    S[_rowv2(parity, (hi >> (j - 4)) & 1, t_loc, j), col] = 1.0
    return S


def _build_selh():
    """[96, 4*96] hi-reduce: 4 variants (group slot in psum accumulation).

    variant v: rows = P rows (12*t_loc + 4*r + hi), col = 24*v + 3*t_loc + r.
    """
    S = np.zeros((96, 4 * 96), np.float32)
    for v in range(4):
        for t_loc in range(TPG):
            for r in range(R):
                for hi in range(NHI):
                    S[12 * t_loc + 4 * r + hi, 96 * v + 24 * v + 3 * t_loc + r] = 1.0
    return S


def _build_resp2(response_core):
    """[128, NG*96]: per group g, rows 16*t_loc+lo, col 12*t_loc+4*r+hi =
    response[8g+t_loc, hi*16+lo, r]."""
    out = np.zeros((128, NG * 96), np.float32)
    for g in range(NG):
        for t_loc in range(TPG):
            t = TPG * g + t_loc
            for hi in range(NHI):
                for r in range(R):
                    out[NLO * t_loc:NLO * t_loc + NLO,
                        96 * g + 12 * t_loc + 4 * r + hi] = \
                        response_core[t, hi * NLO:(hi + 1) * NLO, r]
    return out


# ───────────────────────── generic-path constants (v1) ────────────────────

def _gate_row(t_loc, g):
    """v1 pg-tile row of gate g (= 2d+s) for local tree t_loc."""
    d, s = g // 2, g % 2
    return (64 if s else 0) + 6 * t_loc + d


def _build_sel_generic(path_map):
    pm = np.asarray(path_map).reshape(NLEAF, D)
    sel = np.zeros((4, 128, 128), np.float32)
    for k in range(4):
        for t01 in range(2):
            t_loc = 2 * k + t01
            for leaf in range(NLEAF):
                col = 64 * t01 + leaf
                for j in range(D):
                    sel[k, _gate_row(t_loc, int(pm[leaf, j])), col] += 1.0
    return np.ascontiguousarray(sel.transpose(1, 0, 2).reshape(128, 512))


def _build_rbd_generic(response_core):
    rbd = np.zeros((128, NPAIR * 96), np.float32)
    for p in range(NPAIR):
        q = p % PAIRS_PER_EG
        for t01 in range(2):
            t = 2 * p + t01
            c0 = 96 * p + 6 * q + 3 * t01
            rbd[64 * t01:64 * t01 + 64, c0:c0 + 3] = response_core[t]
    return rbd


# ───────────────────────── program builders ──────────────────────────────

def _patched_act_tables():
    """Force Ln+Exp onto the shared natural_log_exp_and_others table set
    so the ACT LUT isn't reloaded between ln and exp phases."""
    import concourse.bacc as bacc_mod
    from concourse.hw_specs import get_activation_tables as orig

    def patched(arch):
        tabs = orig(arch)
        if "natural_log_exp_and_others" in tabs:
            for name, funcs in tabs.items():
                if name != "natural_log_exp_and_others":
                    funcs.discard(AF.Ln)
                    funcs.discard(AF.Exp)
        return tabs

    class _Ctx:
        def __enter__(self):
            self.saved = bacc_mod.get_activation_tables
            bacc_mod.get_activation_tables = patched

        def __exit__(self, *a):
            bacc_mod.get_activation_tables = self.saved

    return _Ctx()


def _build_program_fast():
    nc = bacc.Bacc("TRN2", target_bir_lowering=False, debug=False,
                   num_devices=NCORES)
    xt = nc.dram_tensor("xt", [F, B], F32R, kind="ExternalInput")
    cws = nc.dram_tensor("cws", [F, NG * MROW], F32R, kind="ExternalInput")
    tbv = nc.dram_tensor("tbv", [128, 4], F32, kind="ExternalInput")
    sel2e = nc.dram_tensor("sel2e", [128, 128], F32R, kind="ExternalInput")
    sel2o = nc.dram_tensor("sel2o", [128, 128], F32R, kind="ExternalInput")
    sel1e = nc.dram_tensor("sel1e", [128, 96], F32R, kind="ExternalInput")
    sel1o = nc.dram_tensor("sel1o", [128, 96], F32R, kind="ExternalInput")
    selh = nc.dram_tensor("selh", [96, 4 * 96], F32R, kind="ExternalInput")
    resp2 = nc.dram_tensor("resp2", [128, NG * 96], F32R, kind="ExternalInput")
    out = nc.dram_tensor("out", [T_C * R, B], F32, kind="ExternalOutput")

    with tile.TileContext(nc) as tc, ExitStack() as ctx:
        cpool = ctx.enter_context(tc.tile_pool(name="consts", bufs=1))
        txt = [cpool.tile([128, B], F32R, name=f"txt{k}", tag=f"xt{k}")
               for k in range(4)]
        tcw = [cpool.tile([128, NG * MROW], F32R, name=f"tcw{k}", tag=f"cw{k}")
               for k in range(4)]
        tb = cpool.tile([128, 4], F32)
        tsel2 = [cpool.tile([128, 128], F32R, name=f"tsel2{p}", tag=f"s2{p}")
                 for p in range(2)]
        tsel1 = [cpool.tile([128, 96], F32R, name=f"tsel1{p}", tag=f"s1{p}")
                 for p in range(2)]
        tselh = cpool.tile([96, 4 * 96], F32R)
        tresp2 = cpool.tile([128, NG * 96], F32R)

        # Critical-path input DMAs on sync/gpsimd only (scalar stays free so
        # the ACT table loads run immediately; tensor stays DMA-free).
        # Const DMAs are emitted later, just before first use, so the
        # fv matmuls' (semaphore-elided) DMA waits don't cover them.
        nc.sync.dma_start(tcw[0][:], cws[0:128, :])
        nc.gpsimd.dma_start(txt[0][:], xt[0:128, :])
        nc.sync.dma_start(txt[1][:], xt[128:256, :])
        nc.gpsimd.dma_start(tcw[1][:], cws[128:256, :])
        nc.sync.dma_start(tcw[2][:], cws[256:384, :])
        nc.gpsimd.dma_start(txt[2][:], xt[256:384, :])
        nc.sync.dma_start(txt[3][:], xt[384:512, :])
        nc.gpsimd.dma_start(tcw[3][:], cws[384:512, :])
        nc.sync.dma_start(tb[:], tbv[:])

        fvpool = ctx.enter_context(tc.tile_pool(name="fvp", bufs=2,
                                                space="PSUM"))
        spool = ctx.enter_context(tc.tile_pool(name="sp", bufs=4,
                                               space="PSUM"))
        opool = ctx.enter_context(tc.tile_pool(name="opp", bufs=2,
                                               space="PSUM"))
        pgpool = ctx.enter_context(tc.tile_pool(name="pgp", bufs=3))
        glpool = ctx.enter_context(tc.tile_pool(name="glp", bufs=3))
        e2pool = ctx.enter_context(tc.tile_pool(name="e2p", bufs=3))
        e1pool = ctx.enter_context(tc.tile_pool(name="e1p", bufs=3))
        pppool = ctx.enter_context(tc.tile_pool(name="ppp", bufs=3))
        evpool = ctx.enter_context(tc.tile_pool(name="evp", bufs=4))

        fv = {}        # (m, nh) -> psum tile
        pg = [None] * NG
        glog = [None] * NG
        e2 = [None] * NG
        e1 = [None] * NG
        pp = [None] * NG
        m1 = {}        # (g, nh)
        op = {}        # (eg, nh)

        def emit_fv(m):
            for nh in range(NH):
                fv[(m, nh)] = fvpool.tile([128, BH], F32, name=f"fv{m}_{nh}",
                                          tag="fv")
            for k in range(4):
                for nh in range(NH):
                    nc.tensor.matmul(fv[(m, nh)][:],
                                     tcw[k][:, 128 * m:128 * (m + 1)],
                                     txt[k][:, BH * nh:BH * (nh + 1)],
                                     start=(k == 0), stop=(k == 3))

        def emit_eltw_ln(g):
            m, par = g // 2, g % 2
            rb = 64 * par                       # fv source row base
            bins = _rowv2(par, 0, 0, 0)         # 0 or 64
            omb = _rowv2(par, 1, 0, 0)          # 64 or 0
            # 64-row ops (48 real + 16 fv-pad rows, which give finite,
            # zero-weighted gate values) so every pg row is written each
            # iteration and all partition starts are 0/64.
            t = pgpool.tile([128, B], F32, name=f"pg{g}", tag="pg")
            pg[g] = t
            gl = glpool.tile([128, B], F32R, name=f"gl{g}", tag="gl")
            glog[g] = gl
            for nh in range(NH):
                hc = slice(BH * nh, BH * (nh + 1))
                src = fv[(m, nh)][rb:rb + 64, :]
                # u = min(fv + b, 1)  (unshifted; per-row b AP)
                nc.vector.tensor_scalar(t[bins:bins + 64, hc], src,
                                        tb[rb:rb + 64, m:m + 1], 1.0,
                                        ALU.add, ALU.min)
                # omb = clamp(1 - u, EPS, 1-EPS)   (shifted, consts only;
                # in-place ops stay on DVE — in-place GPSIMD is ~12x slower)
                nc.gpsimd.tensor_scalar(t[omb:omb + 64, hc],
                                        t[bins:bins + 64, hc],
                                        -1.0, 1.0, ALU.mult, ALU.add)
                nc.vector.tensor_scalar(t[omb:omb + 64, hc],
                                        t[omb:omb + 64, hc],
                                        1.0 - EPS, EPS, ALU.min, ALU.max)
                # bins = max(u, EPS)
                nc.vector.tensor_scalar(t[bins:bins + 64, hc],
                                        t[bins:bins + 64, hc],
                                        EPS, EPS, ALU.max, ALU.max)
                nc.scalar.activation(gl[:, hc], t[:, hc], AF.Ln)

        def emit_s(g):
            par = g % 2
            s2 = [spool.tile([128, BH], F32, name=f"s2_{g}_{nh}", tag="s")
                  for nh in range(NH)]
            s1 = [spool.tile([128, BH], F32, name=f"s1_{g}_{nh}", tag="s")
                  for nh in range(NH)]
            for nh in range(NH):
                nc.tensor.matmul(s2[nh][:], tsel2[par][:],
                                 glog[g][:, BH * nh:BH * (nh + 1)],
                                 start=True, stop=True)
            for nh in range(NH):
                nc.tensor.matmul(s1[nh][0:96, :], tsel1[par][:],
                                 glog[g][:, BH * nh:BH * (nh + 1)],
                                 start=True, stop=True)
            ee2 = e2pool.tile([128, B], F32R, name=f"e2_{g}", tag="e2")
            e2[g] = ee2
            ee1 = e1pool.tile([96, B], F32R, name=f"e1_{g}", tag="e1")
            e1[g] = ee1
            for nh in range(NH):
                nc.scalar.activation(ee2[:, BH * nh:BH * (nh + 1)],
                                     s2[nh][:], AF.Exp)
            for nh in range(NH):
                nc.scalar.activation(ee1[:, BH * nh:BH * (nh + 1)],
                                     s1[nh][0:96, :], AF.Exp)

        def emit_m1(g):
            t = pppool.tile([96, B], F32R, name=f"pp{g}", tag="pp")
            pp[g] = t
            for nh in range(NH):
                m1[(g, nh)] = spool.tile([128, BH], F32, name=f"m1_{g}_{nh}",
                                         tag="s")
                nc.tensor.matmul(m1[(g, nh)][0:96, :],
                                 tresp2[:, 96 * g:96 * (g + 1)],
                                 e2[g][:, BH * nh:BH * (nh + 1)],
                                 start=True, stop=True)
            for nh in range(NH):
                nc.vector.tensor_tensor(t[:, BH * nh:BH * (nh + 1)],
                                        m1[(g, nh)][0:96, :],
                                        e1[g][:, BH * nh:BH * (nh + 1)],
                                        ALU.mult)

        def emit_op(g):
            eg, v = g // 4, g % 4
            if v == 0:
                for nh in range(NH):
                    op[(eg, nh)] = opool.tile([96, BH], F32,
                                              name=f"op{eg}_{nh}", tag="op")
            for nh in range(NH):
                nc.tensor.matmul(op[(eg, nh)][:],
                                 tselh[:, 96 * v:96 * (v + 1)],
                                 pp[g][:, BH * nh:BH * (nh + 1)],
                                 start=(v == 0), stop=(v == 3),
                                 skip_group_check=True)
            if v == 3:
                for nh in range(NH):
                    ev = evpool.tile([96, BH], F32, name=f"ev{eg}_{nh}",
                                     tag="ev")
                    nc.vector.tensor_copy(ev[:], op[(eg, nh)][:])
                    nc.sync.dma_start(
                        out[96 * eg:96 * (eg + 1), BH * nh:BH * (nh + 1)],
                        ev[:])

        # ── pipeline ──
        emit_fv(0)
        emit_fv(1)
        nc.sync.dma_start(tsel2[0][:], sel2e[:])
        nc.gpsimd.dma_start(tsel2[1][:], sel2o[:])
        nc.sync.dma_start(tsel1[0][:], sel1e[:])
        nc.gpsimd.dma_start(tsel1[1][:], sel1o[:])
        for g in range(NG):
            emit_eltw_ln(g)
            emit_s(g)
            if g == 0 and NG > 2:
                emit_fv(2)
                nc.gpsimd.dma_start(tresp2[:], resp2[:])
            if g == 1 and NG > 3:
                emit_fv(3)
                nc.sync.dma_start(tselh[:], selh[:])
            if g >= 1:
                emit_m1(g - 1)
            if g >= 2:
                emit_op(g - 2)
        emit_m1(NG - 1)
        emit_op(NG - 2)
        emit_op(NG - 1)

    with _patched_act_tables():
        nc.compile()
    return nc


# ───────────────────────── generic path (v1, unchanged) ──────────────────

def _common_frontend(nc, tc, ctx):
    """DMA inputs and ecw = exp(feat_attention) tiles."""
    xt = nc.dram_tensor("xt", [F, B + 2], F32R, kind="ExternalInput")
    fap = nc.dram_tensor("fap", [F, NG * MROW], F32, kind="ExternalInput")
    ta0 = nc.dram_tensor("ta0", [128, 4], F32, kind="ExternalInput")
    tbb = nc.dram_tensor("tbb", [128, 4], F32, kind="ExternalInput")

    cpool = ctx.enter_context(tc.tile_pool(name="consts", bufs=1))
    txt = [cpool.tile([128, B + 2], F32R, name=f"txt{k}", tag=f"xt{k}")
           for k in range(4)]
    tfap = [cpool.tile([128, NG * MROW], F32, name=f"tfap{k}", tag=f"fap{k}")
            for k in range(4)]
    tecw = [cpool.tile([128, NG * MROW], F32R, name=f"tecw{k}", tag=f"ecw{k}")
            for k in range(4)]
    tta0 = cpool.tile([128, 4], F32)
    ttb = cpool.tile([128, 4], F32)
    tra = cpool.tile([128, 4], F32)
    trz = cpool.tile([128, 8], F32)

    for k in range(4):
        nc.sync.dma_start(txt[k][:], xt[128 * k:128 * k + 128, :])
        nc.sync.dma_start(tfap[k][:], fap[128 * k:128 * k + 128, :])
    nc.sync.dma_start(tta0[:], ta0[:])
    nc.sync.dma_start(ttb[:], tbb[:])

    for k in range(4):
        nc.scalar.activation(tecw[k][:], tfap[k][:], AF.Exp)

    return cpool, txt, tecw, tra, trz, tta0, ttb


def _emit_glogs(nc, tc, ctx, txt, tecw, tra, trz, tta0, ttb, glog_bufs):
    """Per-group gate-log tiles via M=128 fv matmuls with fused Z columns.
    Returns list of glog APs."""
    glpool = ctx.enter_context(tc.tile_pool(name="glp", bufs=glog_bufs))
    lctx = ctx.enter_context(ExitStack())
    fvpool = lctx.enter_context(tc.tile_pool(name="fvps", bufs=1, space="PSUM"))
    wpool = lctx.enter_context(tc.tile_pool(name="work", bufs=2))
    pgpool = lctx.enter_context(tc.tile_pool(name="pgp", bufs=2))
    glogs = [None] * NG
    for m in range(4):          # M-tile = 2 gate groups (2m, 2m+1)
        fv = fvpool.tile([128, B + 2], F32, name=f"fv{m}", tag="fv")
        for k in range(4):
            for off, n in ((0, 512), (512, 512), (1024, 2)):
                nc.tensor.matmul(fv[:, off:off + n],
                                 tecw[k][:, 128 * m:128 * (m + 1)],
                                 txt[k][:, off:off + n],
                                 start=(k == 0), stop=(k == 3))
        nc.vector.reciprocal(trz[:, 2 * m:2 * m + 2], fv[:, 1024:1026])
        nc.vector.tensor_mul(tra[:, m:m + 1], tta0[:, m:m + 1],
                             trz[:, 2 * m:2 * m + 1])
        tmp = wpool.tile([128, B], F32, name=f"tmp{m}", tag="tmp")
        nc.vector.tensor_scalar(tmp[:], fv[:, 0:1024], tra[:, m:m + 1],
                                ttb[:, m:m + 1], ALU.mult, ALU.add)
        for half in range(2):
            g = 2 * m + half
            th = tmp[64 * half:64 * half + 64, :]
            pgt = pgpool.tile([128, B], F32R, name=f"pg{g}", tag="pg")
            nc.gpsimd.tensor_scalar(pgt[0:64, :], th, 1.0, EPS, ALU.min, ALU.max)
            nc.gpsimd.tensor_scalar(pgt[64:128, :], th, -1.0, 1.0,
                                    ALU.mult, ALU.add)
            nc.vector.tensor_scalar(pgt[64:128, :], pgt[64:128, :], 1.0 - EPS,
                                    EPS, ALU.min, ALU.max)
            gl = glpool.tile([128, B], F32R, name=f"glog{g}", tag="glog")
            nc.scalar.activation(gl[:], pgt[:], AF.Ln)
            glogs[g] = gl
    lctx.close()
    return glogs


def _build_program_generic():
    nc = bacc.Bacc("TRN2", target_bir_lowering=False, debug=False,
                   num_devices=NCORES)
    selz = nc.dram_tensor("selz", [128, 512], F32R, kind="ExternalInput")
    rbd = nc.dram_tensor("rbd", [128, NPAIR * 96], F32R, kind="ExternalInput")
    out = nc.dram_tensor("out", [T_C * R, B], F32, kind="ExternalOutput")

    with tile.TileContext(nc) as tc, ExitStack() as ctx:
        cpool, txt, tecw, tra, trz, tta0, ttb = _common_frontend(nc, tc, ctx)
        tselz = cpool.tile([128, 512], F32R)
        trbd = cpool.tile([128, NPAIR * 96], F32R)
        nc.sync.dma_start(tselz[:], selz[:])
        nc.sync.dma_start(trbd[:], rbd[:])

        glogs = _emit_glogs(nc, tc, ctx, txt, tecw, tra, trz, tta0, ttb,
                            glog_bufs=3)

        rwpool = ctx.enter_context(tc.tile_pool(name="rwp", bufs=3))
        evpool = ctx.enter_context(tc.tile_pool(name="evp", bufs=2))
        with (
            tc.tile_pool(name="sps", bufs=2, space="PSUM") as spool,
            tc.tile_pool(name="ops", bufs=1, space="PSUM") as opool,
        ):
            for eg in range(2):
                opt = opool.tile([96, B], F32, name=f"op{eg}", tag="outp")
                for gi in range(NG // 2):
                    g = eg * (NG // 2) + gi
                    for k in range(4):
                        p = 4 * g + k
                        q = p % PAIRS_PER_EG
                        sp = spool.tile([128, B], F32, name=f"sp{p}", tag="s")
                        for nh in range(NH):
                            nc.tensor.matmul(sp[:, 512 * nh:512 * (nh + 1)],
                                             tselz[:, 128 * k:128 * (k + 1)],
                                             glogs[g][:, 512 * nh:512 * (nh + 1)],
                                             start=True, stop=True)
                        rw = rwpool.tile([128, B], F32R, name=f"rw{p}", tag="rw")
                        nc.scalar.activation(rw[:], sp[:], AF.Exp)
                        for nh in range(NH):
                            nc.tensor.matmul(opt[:, 512 * nh:512 * (nh + 1)],
                                             trbd[:, 96 * p:96 * (p + 1)],
                                             rw[:, 512 * nh:512 * (nh + 1)],
                                             start=(q == 0),
                                             stop=(q == PAIRS_PER_EG - 1),
                                             skip_group_check=True)
                ev = evpool.tile([96, B], F32, name=f"ev{eg}", tag="ev")
                nc.vector.tensor_copy(ev[:], opt[:])
                nc.sync.dma_start(out[96 * eg:96 * (eg + 1), :], ev[:])

    with _patched_act_tables():
        nc.compile()
    return nc


# ───────────────────────── host entry point ──────────────────────────────

def _host_prep_fast(x, feat_attention, thresholds, log_temperatures):
    """Shared (core-independent) fast-path prep: xt, folded weights, b."""
    xto = np.ascontiguousarray(x.T)
    ecw = np.exp(feat_attention)
    cw = ecw / ecw.sum(axis=0, keepdims=True)          # softmax over features
    elt = np.exp(-log_temperatures)                    # [T, D]
    a0 = 0.5 * elt
    cw_s = cw.reshape(F, T, D) * a0[None]              # fold scale into weights
    cw_s = cw_s.reshape(F, T * D)
    b_all = 0.5 - 0.5 * thresholds * elt               # [T, D]
    return xto, cw_s, b_all


def _host_prep_fast_core(c, cw_s, b_all):
    t0 = T_C * c
    cw_c = cw_s[:, D * t0: D * (t0 + T_C)]             # [F, 384]
    cws = np.zeros((F, NG * MROW), np.float32)
    tbv = np.zeros((128, 4), np.float32)
    for g in range(NG):
        m, par = g // 2, g % 2
        cws[:, 128 * m + 64 * par: 128 * m + 64 * par + 48] = \
            cw_c[:, 48 * g: 48 * g + 48]
        for t_loc in range(TPG):
            t = t0 + TPG * g + t_loc
            rows = slice(64 * par + 6 * t_loc, 64 * par + 6 * t_loc + 6)
            tbv[rows, m] = b_all[t]
    return dict(cws=cws, tbv=tbv)


def _host_prep_core(c, xto, feat_attention, a0_all, b_all):
    t0 = T_C * c
    fa_c = feat_attention[:, D * t0: D * (t0 + T_C)]
    fap = np.zeros((F, NG * MROW), np.float32)
    ta0 = np.zeros((128, 4), np.float32)
    tbb = np.full((128, 4), 0.5, np.float32)
    for g in range(NG):
        fap[:, MROW * g: MROW * g + 48] = fa_c[:, 48 * g: 48 * g + 48]
        m, half = g // 2, g % 2
        for t_loc in range(TPG):
            t = t0 + TPG * g + t_loc
            rows = slice(64 * half + 6 * t_loc, 64 * half + 6 * t_loc + 6)
            ta0[rows, m] = a0_all[t]
            tbb[rows, m] = b_all[t]
    return dict(xt=xto, fap=fap, ta0=ta0, tbb=tbb)


def _enable_ldw_opt():
    """Turn on walrus's LDWEIGHTS dedup for this process's compiles
    (validated: identical results, fewer redundant weight loads)."""
    import concourse.bass_utils as bu
    if getattr(bu.run_command, "_ldw_opt", False):
        return
    orig = bu.run_command

    def patched(argv, **kw):
        argv = [a.replace("--enable-ldw-opt=false", "--enable-ldw-opt=true")
                for a in argv]
        return orig(argv, **kw)

    patched._ldw_opt = True
    bu.run_command = patched


def kernel(x, feat_attention, thresholds, log_temperatures, response, path_map):
    _enable_ldw_opt()
    x = np.ascontiguousarray(np.asarray(x, dtype=np.float32))
    feat_attention = np.asarray(feat_attention, dtype=np.float32)
    thresholds = np.asarray(thresholds, dtype=np.float32)
    log_temperatures = np.asarray(log_temperatures, dtype=np.float32)
    response = np.asarray(response, dtype=np.float32)

    fast = _is_oblivious(path_map)
    key = "fast" if fast else "generic"
    if key not in _CACHE:
        _CACHE[key] = (_build_program_fast() if fast
                       else _build_program_generic())
    nc = _CACHE[key]

    in_maps = []
    if fast:
        xto, cw_s, b_all = _host_prep_fast(x, feat_attention, thresholds,
                                           log_temperatures)
        for c in range(NCORES):
            m = _host_prep_fast_core(c, cw_s, b_all)
            m["xt"] = xto
            m["sel2e"] = _CACHE.setdefault("sel2e", _build_sel2_v2(0))
            m["sel2o"] = _CACHE.setdefault("sel2o", _build_sel2_v2(1))
            m["sel1e"] = _CACHE.setdefault("sel1e", _build_sel1_v2(0))
            m["sel1o"] = _CACHE.setdefault("sel1o", _build_sel1_v2(1))
            m["selh"] = _CACHE.setdefault("selh", _build_selh())
            m["resp2"] = _build_resp2(response[T_C * c:T_C * (c + 1)])
            in_maps.append(m)
    else:
        xto = np.ascontiguousarray(
            np.concatenate([x.T, np.ones((F, 2), np.float32)], axis=1))
        elt = np.exp(-log_temperatures)
        a0_all = 0.5 * elt
        b_all = 0.5 - 0.5 * thresholds * elt
        for c in range(NCORES):
            m = _host_prep_core(c, xto, feat_attention, a0_all, b_all)
            t0 = T_C * c
            if "selg" not in _CACHE:
                _CACHE["selg"] = _build_sel_generic(path_map)
            m["selz"] = _CACHE["selg"]
            m["rbd"] = _build_rbd_generic(response[t0:t0 + T_C])
            in_maps.append(m)

    _CACHE["in_maps"] = in_maps
    res = run_bass_kernel_spmd(nc, in_maps, core_ids=list(range(NCORES)))
    outs = [res.results[c]["out"].T for c in range(NCORES)]
    return np.ascontiguousarray(np.concatenate(outs, axis=1))


# revision 15
# speedup vs baseline: 1.0123x; 1.0123x over previous
"""Trainium2 Bass kernel for nn_DeTree (NODE-style oblivious decision ensemble).

Tree-sharded over 8 cores (64 trees/core), full batch per core.

Fast path (oblivious path_map), v2 — fully software-pipelined:
  Host folds softmax(feat_attention), the temperature scale (0.5*exp(-lt))
  and the softmax denominator into the fv weights `cws`, so the device
  pipeline is pure matmul/elementwise/act with no frontend exp:
    1. PE: fv(m,nh) = cws[:,m-block]^T @ x^T[, nh-half]   (f32r)
    2. DVE/GPSIMD: u = min(fv+b, 1); bins = max(u, EPS);
       omb = clamp(1-u, EPS, 1-EPS)  (pg tile, parity-swapped row layout)
    3. ACT: glog = ln(pg)
    4. PE: lo-sums S2 (16 combos/tree) + replicated hi-sums S1
       (3r x 4 combos/tree) via constant 0/1 selection matmuls.
    5. ACT: E2 = exp(S2), E1 = exp(S1)
    6. PE: M1[t,(r,hi)] = sum_lo resp[t,hi*16+lo,r] * E2[t,lo] (block-diag)
    7. DVE: P = M1 * E1
    8. PE: out[t*3+r] = sum_hi P, 4 groups accumulated per psum tile.
  All stages interleave per tree-group g (PE order: S(g), M1(g-1), OP(g-2))
  so PE/ACT/DVE/GPSIMD stay busy concurrently; input DMAs are spread
  across engine queues so the first fv matmul starts ~1us in.
Generic path (any path_map): 2-trees-per-matmul leaf log-sum (64 leaves),
exp, response block-diag accumulation (v1, unchanged).
"""
import numpy as np
from contextlib import ExitStack

import concourse.bass as bass
import concourse.bacc as bacc
import concourse.tile as tile
import concourse.mybir as mybir
from concourse.bass_utils import run_bass_kernel_spmd

F32 = mybir.dt.float32
F32R = mybir.dt.float32r
AF = mybir.ActivationFunctionType
ALU = mybir.AluOpType

B = 1024          # batch
F = 512           # in_features
T = 512           # num_trees
D = 6             # depth
R = 3             # response_dim
NLEAF = 64
NCORES = 8
T_C = T // NCORES          # 64 trees per core
TPG = 8                    # trees per gate-tile group
NG = T_C // TPG            # 8 groups per core
MROW = 64                  # padded rows per fv M-tile half (48 real + 16 pad)
NPAIR = T_C // 2           # generic path: 32 tree-pairs per core
PAIRS_PER_EG = 16
EPS = 2.0 ** -20
NH = 2                     # batch halves (1024 = 2 x 512)
BH = B // NH               # 512
NLO = 16                   # 2^4 lo-combos (depths 0..3)
NHI = 4                    # 2^2 hi-combos (depths 4..5)

_CACHE = {}


def _is_oblivious(path_map):
    pm = np.asarray(path_map).reshape(NLEAF, D)
    exp = np.array([[2 * j + ((l >> j) & 1) for j in range(D)]
                    for l in range(NLEAF)], dtype=pm.dtype)
    return bool(np.array_equal(pm, exp))


# ───────────────────────── fast (v2) constants ────────────────────────────
# pg row layout per group parity (within its [128, B] tile):
#   even g: bins rows 0..47,  omb rows 64..111  (src fv partitions 0..47)
#   odd  g: bins rows 64..111, omb rows 0..47   (src fv partitions 64..111)
# The bins write is partition-UNSHIFTED from fv and carries the per-row
# b-vector AP; the omb write derives from bins with constant scalars only.
# All partition starts are 0/64 (hardware requires starts in {0,32,64,96});
# rows 48..63 are never written (memset once per ring buffer to stay
# ln-safe), rows 112..127 are never read.

def _rowv2(parity, s, t_loc, d):
    """pg-tile row of gate (d, s) for local tree t_loc. s=0: bins, s=1: omb."""
    if parity == 0:
        base = 0 if s == 0 else 64
    else:
        base = 64 if s == 0 else 0
    return base + 6 * t_loc + d


def _build_sel2_v2(parity):
    """[128, 128] lo-sum selection: col = 16*t_loc + lo, depths 0..3."""
    S = np.zeros((128, 128), np.float32)
    for t_loc in range(TPG):
        for lo in range(NLO):
            col = NLO * t_loc + lo
            for j in range(4):
                S[_rowv2(parity, (lo >> j) & 1, t_loc, j), col] = 1.0
    return S


def _build_sel1_v2(parity):
    """[128, 96] replicated hi-sum selection: col = 12*t_loc + 4*r + hi."""
    S = np.zeros((128, 96), np.float32)
    for t_loc in range(TPG):
        for r in range(R):
            for hi in range(NHI):
                col = 12 * t_loc + 4 * r + hi
                for j in range(4, 6):
                    S[_rowv2(parity, (hi >> (j - 4)) & 1, t_loc, j), col] = 1.0
    return S


def _build_selh():
    """[96, 4*96] hi-reduce: 4 variants (group slot in psum accumulation).

    variant v: rows = P rows (12*t_loc + 4*r + hi), col = 24*v + 3*t_loc + r.
    """
    S = np.zeros((96, 4 * 96), np.float32)
    for v in range(4):
        for t_loc in range(TPG):
            for r in range(R):
                for hi in range(NHI):
                    S[12 * t_loc + 4 * r + hi, 96 * v + 24 * v + 3 * t_loc + r] = 1.0
    return S


def _build_resp2(response_core):
    """[128, NG*96]: per group g, rows 16*t_loc+lo, col 12*t_loc+4*r+hi =
    response[8g+t_loc, hi*16+lo, r]."""
    out = np.zeros((128, NG * 96), np.float32)
    for g in range(NG):
        for t_loc in range(TPG):
            t = TPG * g + t_loc
            for hi in range(NHI):
                for r in range(R):
                    out[NLO * t_loc:NLO * t_loc + NLO,
                        96 * g + 12 * t_loc + 4 * r + hi] = \
                        response_core[t, hi * NLO:(hi + 1) * NLO, r]
    return out


# ───────────────────────── generic-path constants (v1) ────────────────────

def _gate_row(t_loc, g):
    """v1 pg-tile row of gate g (= 2d+s) for local tree t_loc."""
    d, s = g // 2, g % 2
    return (64 if s else 0) + 6 * t_loc + d


def _build_sel_generic(path_map):
    pm = np.asarray(path_map).reshape(NLEAF, D)
    sel = np.zeros((4, 128, 128), np.float32)
    for k in range(4):
        for t01 in range(2):
            t_loc = 2 * k + t01
            for leaf in range(NLEAF):
                col = 64 * t01 + leaf
                for j in range(D):
                    sel[k, _gate_row(t_loc, int(pm[leaf, j])), col] += 1.0
    return np.ascontiguousarray(sel.transpose(1, 0, 2).reshape(128, 512))


def _build_rbd_generic(response_core):
    rbd = np.zeros((128, NPAIR * 96), np.float32)
    for p in range(NPAIR):
        q = p % PAIRS_PER_EG
        for t01 in range(2):
            t = 2 * p + t01
            c0 = 96 * p + 6 * q + 3 * t01
            rbd[64 * t01:64 * t01 + 64, c0:c0 + 3] = response_core[t]
    return rbd


# ───────────────────────── program builders ──────────────────────────────

def _patched_act_tables():
    """Force Ln+Exp onto the shared natural_log_exp_and_others table set
    so the ACT LUT isn't reloaded between ln and exp phases."""
    import concourse.bacc as bacc_mod
    from concourse.hw_specs import get_activation_tables as orig

    def patched(arch):
        tabs = orig(arch)
        if "natural_log_exp_and_others" in tabs:
            for name, funcs in tabs.items():
                if name != "natural_log_exp_and_others":
                    funcs.discard(AF.Ln)
                    funcs.discard(AF.Exp)
        return tabs

    class _Ctx:
        def __enter__(self):
            self.saved = bacc_mod.get_activation_tables
            bacc_mod.get_activation_tables = patched

        def __exit__(self, *a):
            bacc_mod.get_activation_tables = self.saved

    return _Ctx()


def _build_program_fast():
    nc = bacc.Bacc("TRN2", target_bir_lowering=False, debug=False,
                   num_devices=NCORES)
    xt = nc.dram_tensor("xt", [F, B], F32R, kind="ExternalInput")
    cws = nc.dram_tensor("cws", [F, NG * MROW], F32R, kind="ExternalInput")
    tbv = nc.dram_tensor("tbv", [128, 4], F32, kind="ExternalInput")
    sel2e = nc.dram_tensor("sel2e", [128, 128], F32R, kind="ExternalInput")
    sel2o = nc.dram_tensor("sel2o", [128, 128], F32R, kind="ExternalInput")
    sel1e = nc.dram_tensor("sel1e", [128, 96], F32R, kind="ExternalInput")
    sel1o = nc.dram_tensor("sel1o", [128, 96], F32R, kind="ExternalInput")
    selh = nc.dram_tensor("selh", [96, 4 * 96], F32R, kind="ExternalInput")
    resp2 = nc.dram_tensor("resp2", [128, NG * 96], F32R, kind="ExternalInput")
    out = nc.dram_tensor("out", [T_C * R, B], F32, kind="ExternalOutput")

    with tile.TileContext(nc) as tc, ExitStack() as ctx:
        cpool = ctx.enter_context(tc.tile_pool(name="consts", bufs=1))
        txt = [cpool.tile([128, B], F32R, name=f"txt{k}", tag=f"xt{k}")
               for k in range(4)]
        tcw = [cpool.tile([128, NG * MROW], F32R, name=f"tcw{k}", tag=f"cw{k}")
               for k in range(4)]
        tb = cpool.tile([128, 4], F32)
        tsel2 = [cpool.tile([128, 128], F32R, name=f"tsel2{p}", tag=f"s2{p}")
                 for p in range(2)]
        tsel1 = [cpool.tile([128, 96], F32R, name=f"tsel1{p}", tag=f"s1{p}")
                 for p in range(2)]
        tselh = cpool.tile([96, 4 * 96], F32R)
        tresp2 = cpool.tile([128, NG * 96], F32R)

        # Input DMAs spread across the three DMA-capable queues
        # (sync/scalar/gpsimd); the pair (cw[k], xt[k]) needed first lands
        # first. Tensor queue stays DMA-free.
        nc.sync.dma_start(tcw[0][:], cws[0:128, :])
        nc.scalar.dma_start(txt[0][:], xt[0:128, :])
        nc.gpsimd.dma_start(tcw[3][:], cws[384:512, :])
        nc.sync.dma_start(txt[1][:], xt[128:256, :])
        nc.scalar.dma_start(tcw[1][:], cws[128:256, :])
        nc.gpsimd.dma_start(txt[2][:], xt[256:384, :])
        nc.sync.dma_start(tcw[2][:], cws[256:384, :])
        nc.scalar.dma_start(tb[:], tbv[:])
        nc.gpsimd.dma_start(tsel2[1][:], sel2o[:])
        nc.sync.dma_start(txt[3][:], xt[384:512, :])
        nc.scalar.dma_start(tresp2[:], resp2[:])
        nc.gpsimd.dma_start(tsel1[1][:], sel1o[:])
        nc.sync.dma_start(tsel2[0][:], sel2e[:])
        nc.gpsimd.dma_start(tselh[:], selh[:])
        nc.sync.dma_start(tsel1[0][:], sel1e[:])

        fvpool = ctx.enter_context(tc.tile_pool(name="fvp", bufs=3,
                                                space="PSUM"))
        spool = ctx.enter_context(tc.tile_pool(name="sp", bufs=3,
                                               space="PSUM"))
        opool = ctx.enter_context(tc.tile_pool(name="opp", bufs=2,
                                               space="PSUM"))
        pgpool = ctx.enter_context(tc.tile_pool(name="pgp", bufs=3))
        glpool = ctx.enter_context(tc.tile_pool(name="glp", bufs=3))
        e2pool = ctx.enter_context(tc.tile_pool(name="e2p", bufs=3))
        e1pool = ctx.enter_context(tc.tile_pool(name="e1p", bufs=3))
        pppool = ctx.enter_context(tc.tile_pool(name="ppp", bufs=3))
        evpool = ctx.enter_context(tc.tile_pool(name="evp", bufs=4))

        fv = {}        # (m, nh) -> psum tile
        pg = [None] * NG
        glog = [None] * NG
        e2 = [None] * NG
        e1 = [None] * NG
        pp = [None] * NG
        m1 = {}        # (g, nh)
        op = {}        # (eg, nh)

        def emit_fv(m):
            for nh in range(NH):
                fv[(m, nh)] = fvpool.tile([128, BH], F32, name=f"fv{m}_{nh}",
                                          tag="fv")
            for k in range(4):
                for nh in range(NH):
                    nc.tensor.matmul(fv[(m, nh)][:],
                                     tcw[k][:, 128 * m:128 * (m + 1)],
                                     txt[k][:, BH * nh:BH * (nh + 1)],
                                     start=(k == 0), stop=(k == 3))

        def emit_eltw_ln(g):
            m, par = g // 2, g % 2
            rb = 64 * par                       # fv source row base
            bins = _rowv2(par, 0, 0, 0)         # 0 or 64
            omb = _rowv2(par, 1, 0, 0)          # 64 or 0
            # 64-row ops (48 real + 16 fv-pad rows, which give finite,
            # zero-weighted gate values) so every pg row is written each
            # iteration and all partition starts are 0/64.
            t = pgpool.tile([128, B], F32, name=f"pg{g}", tag="pg")
            pg[g] = t
            gl = glpool.tile([128, B], F32R, name=f"gl{g}", tag="gl")
            glog[g] = gl
            for nh in range(NH):
                hc = slice(BH * nh, BH * (nh + 1))
                src = fv[(m, nh)][rb:rb + 64, :]
                # u = min(fv + b, 1)  (unshifted; per-row b AP)
                nc.vector.tensor_scalar(t[bins:bins + 64, hc], src,
                                        tb[rb:rb + 64, m:m + 1], 1.0,
                                        ALU.add, ALU.min)
                # omb = clamp(1 - u, EPS, 1-EPS)   (shifted, consts only;
                # in-place ops stay on DVE — in-place GPSIMD is ~12x slower)
                nc.gpsimd.tensor_scalar(t[omb:omb + 64, hc],
                                        t[bins:bins + 64, hc],
                                        -1.0, 1.0, ALU.mult, ALU.add)
                nc.vector.tensor_scalar(t[omb:omb + 64, hc],
                                        t[omb:omb + 64, hc],
                                        1.0 - EPS, EPS, ALU.min, ALU.max)
                # bins = max(u, EPS)
                nc.vector.tensor_scalar(t[bins:bins + 64, hc],
                                        t[bins:bins + 64, hc],
                                        EPS, EPS, ALU.max, ALU.max)
                nc.scalar.activation(gl[:, hc], t[:, hc], AF.Ln)

        def emit_s(g):
            par = g % 2
            s2 = [spool.tile([128, BH], F32, name=f"s2_{g}_{nh}", tag="s")
                  for nh in range(NH)]
            s1 = [spool.tile([128, BH], F32, name=f"s1_{g}_{nh}", tag="s")
                  for nh in range(NH)]
            for nh in range(NH):
                nc.tensor.matmul(s2[nh][:], tsel2[par][:],
                                 glog[g][:, BH * nh:BH * (nh + 1)],
                                 start=True, stop=True)
            for nh in range(NH):
                nc.tensor.matmul(s1[nh][0:96, :], tsel1[par][:],
                                 glog[g][:, BH * nh:BH * (nh + 1)],
                                 start=True, stop=True)
            ee2 = e2pool.tile([128, B], F32R, name=f"e2_{g}", tag="e2")
            e2[g] = ee2
            ee1 = e1pool.tile([96, B], F32R, name=f"e1_{g}", tag="e1")
            e1[g] = ee1
            for nh in range(NH):
                nc.scalar.activation(ee2[:, BH * nh:BH * (nh + 1)],
                                     s2[nh][:], AF.Exp)
            for nh in range(NH):
                nc.scalar.activation(ee1[:, BH * nh:BH * (nh + 1)],
                                     s1[nh][0:96, :], AF.Exp)

        def emit_m1(g):
            t = pppool.tile([96, B], F32R, name=f"pp{g}", tag="pp")
            pp[g] = t
            for nh in range(NH):
                m1[(g, nh)] = spool.tile([128, BH], F32, name=f"m1_{g}_{nh}",
                                         tag="s")
                nc.tensor.matmul(m1[(g, nh)][0:96, :],
                                 tresp2[:, 96 * g:96 * (g + 1)],
                                 e2[g][:, BH * nh:BH * (nh + 1)],
                                 start=True, stop=True)
            for nh in range(NH):
                nc.vector.tensor_tensor(t[:, BH * nh:BH * (nh + 1)],
                                        m1[(g, nh)][0:96, :],
                                        e1[g][:, BH * nh:BH * (nh + 1)],
                                        ALU.mult)

        def emit_op(g):
            eg, v = g // 4, g % 4
            if v == 0:
                for nh in range(NH):
                    op[(eg, nh)] = opool.tile([96, BH], F32,
                                              name=f"op{eg}_{nh}", tag="op")
            for nh in range(NH):
                nc.tensor.matmul(op[(eg, nh)][:],
                                 tselh[:, 96 * v:96 * (v + 1)],
                                 pp[g][:, BH * nh:BH * (nh + 1)],
                                 start=(v == 0), stop=(v == 3),
                                 skip_group_check=True)
            if v == 3:
                for nh in range(NH):
                    ev = evpool.tile([96, BH], F32, name=f"ev{eg}_{nh}",
                                     tag="ev")
                    nc.vector.tensor_copy(ev[:], op[(eg, nh)][:])
                    nc.sync.dma_start(
                        out[96 * eg:96 * (eg + 1), BH * nh:BH * (nh + 1)],
                        ev[:])

        # ── pipeline ──
        emit_fv(0)
        emit_fv(1)
        for g in range(NG):
            emit_eltw_ln(g)
            emit_s(g)
            if g == 0 and NG > 2:
                emit_fv(2)
            if g == 1 and NG > 3:
                emit_fv(3)
            if g >= 1:
                emit_m1(g - 1)
            if g >= 2:
                emit_op(g - 2)
        emit_m1(NG - 1)
        emit_op(NG - 2)
        emit_op(NG - 1)

    with _patched_act_tables():
        nc.compile()
    return nc


# ───────────────────────── generic path (v1, unchanged) ──────────────────

def _common_frontend(nc, tc, ctx):
    """DMA inputs and ecw = exp(feat_attention) tiles."""
    xt = nc.dram_tensor("xt", [F, B + 2], F32R, kind="ExternalInput")
    fap = nc.dram_tensor("fap", [F, NG * MROW], F32, kind="ExternalInput")
    ta0 = nc.dram_tensor("ta0", [128, 4], F32, kind="ExternalInput")
    tbb = nc.dram_tensor("tbb", [128, 4], F32, kind="ExternalInput")

    cpool = ctx.enter_context(tc.tile_pool(name="consts", bufs=1))
    txt = [cpool.tile([128, B + 2], F32R, name=f"txt{k}", tag=f"xt{k}")
           for k in range(4)]
    tfap = [cpool.tile([128, NG * MROW], F32, name=f"tfap{k}", tag=f"fap{k}")
            for k in range(4)]
    tecw = [cpool.tile([128, NG * MROW], F32R, name=f"tecw{k}", tag=f"ecw{k}")
            for k in range(4)]
    tta0 = cpool.tile([128, 4], F32)
    ttb = cpool.tile([128, 4], F32)
    tra = cpool.tile([128, 4], F32)
    trz = cpool.tile([128, 8], F32)

    for k in range(4):
        nc.sync.dma_start(txt[k][:], xt[128 * k:128 * k + 128, :])
        nc.sync.dma_start(tfap[k][:], fap[128 * k:128 * k + 128, :])
    nc.sync.dma_start(tta0[:], ta0[:])
    nc.sync.dma_start(ttb[:], tbb[:])

    for k in range(4):
        nc.scalar.activation(tecw[k][:], tfap[k][:], AF.Exp)

    return cpool, txt, tecw, tra, trz, tta0, ttb


def _emit_glogs(nc, tc, ctx, txt, tecw, tra, trz, tta0, ttb, glog_bufs):
    """Per-group gate-log tiles via M=128 fv matmuls with fused Z columns.
    Returns list of glog APs."""
    glpool = ctx.enter_context(tc.tile_pool(name="glp", bufs=glog_bufs))
    lctx = ctx.enter_context(ExitStack())
    fvpool = lctx.enter_context(tc.tile_pool(name="fvps", bufs=1, space="PSUM"))
    wpool = lctx.enter_context(tc.tile_pool(name="work", bufs=2))
    pgpool = lctx.enter_context(tc.tile_pool(name="pgp", bufs=2))
    glogs = [None] * NG
    for m in range(4):          # M-tile = 2 gate groups (2m, 2m+1)
        fv = fvpool.tile([128, B + 2], F32, name=f"fv{m}", tag="fv")
        for k in range(4):
            for off, n in ((0, 512), (512, 512), (1024, 2)):
                nc.tensor.matmul(fv[:, off:off + n],
                                 tecw[k][:, 128 * m:128 * (m + 1)],
                                 txt[k][:, off:off + n],
                                 start=(k == 0), stop=(k == 3))
        nc.vector.reciprocal(trz[:, 2 * m:2 * m + 2], fv[:, 1024:1026])
        nc.vector.tensor_mul(tra[:, m:m + 1], tta0[:, m:m + 1],
                             trz[:, 2 * m:2 * m + 1])
        tmp = wpool.tile([128, B], F32, name=f"tmp{m}", tag="tmp")
        nc.vector.tensor_scalar(tmp[:], fv[:, 0:1024], tra[:, m:m + 1],
                                ttb[:, m:m + 1], ALU.mult, ALU.add)
        for half in range(2):
            g = 2 * m + half
            th = tmp[64 * half:64 * half + 64, :]
            pgt = pgpool.tile([128, B], F32R, name=f"pg{g}", tag="pg")
            nc.gpsimd.tensor_scalar(pgt[0:64, :], th, 1.0, EPS, ALU.min, ALU.max)
            nc.gpsimd.tensor_scalar(pgt[64:128, :], th, -1.0, 1.0,
                                    ALU.mult, ALU.add)
            nc.vector.tensor_scalar(pgt[64:128, :], pgt[64:128, :], 1.0 - EPS,
                                    EPS, ALU.min, ALU.max)
            gl = glpool.tile([128, B], F32R, name=f"glog{g}", tag="glog")
            nc.scalar.activation(gl[:], pgt[:], AF.Ln)
            glogs[g] = gl
    lctx.close()
    return glogs


def _build_program_generic():
    nc = bacc.Bacc("TRN2", target_bir_lowering=False, debug=False,
                   num_devices=NCORES)
    selz = nc.dram_tensor("selz", [128, 512], F32R, kind="ExternalInput")
    rbd = nc.dram_tensor("rbd", [128, NPAIR * 96], F32R, kind="ExternalInput")
    out = nc.dram_tensor("out", [T_C * R, B], F32, kind="ExternalOutput")

    with tile.TileContext(nc) as tc, ExitStack() as ctx:
        cpool, txt, tecw, tra, trz, tta0, ttb = _common_frontend(nc, tc, ctx)
        tselz = cpool.tile([128, 512], F32R)
        trbd = cpool.tile([128, NPAIR * 96], F32R)
        nc.sync.dma_start(tselz[:], selz[:])
        nc.sync.dma_start(trbd[:], rbd[:])

        glogs = _emit_glogs(nc, tc, ctx, txt, tecw, tra, trz, tta0, ttb,
                            glog_bufs=3)

        rwpool = ctx.enter_context(tc.tile_pool(name="rwp", bufs=3))
        evpool = ctx.enter_context(tc.tile_pool(name="evp", bufs=2))
        with (
            tc.tile_pool(name="sps", bufs=2, space="PSUM") as spool,
            tc.tile_pool(name="ops", bufs=1, space="PSUM") as opool,
        ):
            for eg in range(2):
                opt = opool.tile([96, B], F32, name=f"op{eg}", tag="outp")
                for gi in range(NG // 2):
                    g = eg * (NG // 2) + gi
                    for k in range(4):
                        p = 4 * g + k
                        q = p % PAIRS_PER_EG
                        sp = spool.tile([128, B], F32, name=f"sp{p}", tag="s")
                        for nh in range(NH):
                            nc.tensor.matmul(sp[:, 512 * nh:512 * (nh + 1)],
                                             tselz[:, 128 * k:128 * (k + 1)],
                                             glogs[g][:, 512 * nh:512 * (nh + 1)],
                                             start=True, stop=True)
                        rw = rwpool.tile([128, B], F32R, name=f"rw{p}", tag="rw")
                        nc.scalar.activation(rw[:], sp[:], AF.Exp)
                        for nh in range(NH):
                            nc.tensor.matmul(opt[:, 512 * nh:512 * (nh + 1)],
                                             trbd[:, 96 * p:96 * (p + 1)],
                                             rw[:, 512 * nh:512 * (nh + 1)],
                                             start=(q == 0),
                                             stop=(q == PAIRS_PER_EG - 1),
                                             skip_group_check=True)
                ev = evpool.tile([96, B], F32, name=f"ev{eg}", tag="ev")
                nc.vector.tensor_copy(ev[:], opt[:])
                nc.sync.dma_start(out[96 * eg:96 * (eg + 1), :], ev[:])

    with _patched_act_tables():
        nc.compile()
    return nc


# ───────────────────────── host entry point ──────────────────────────────

def _host_prep_fast(x, feat_attention, thresholds, log_temperatures):
    """Shared (core-independent) fast-path prep: xt, folded weights, b."""
    xto = np.ascontiguousarray(x.T)
    ecw = np.exp(feat_attention)
    cw = ecw / ecw.sum(axis=0, keepdims=True)          # softmax over features
    elt = np.exp(-log_temperatures)                    # [T, D]
    a0 = 0.5 * elt
    cw_s = cw.reshape(F, T, D) * a0[None]              # fold scale into weights
    cw_s = cw_s.reshape(F, T * D)
    b_all = 0.5 - 0.5 * thresholds * elt               # [T, D]
    return xto, cw_s, b_all


def _host_prep_fast_core(c, cw_s, b_all):
    t0 = T_C * c
    cw_c = cw_s[:, D * t0: D * (t0 + T_C)]             # [F, 384]
    cws = np.zeros((F, NG * MROW), np.float32)
    tbv = np.zeros((128, 4), np.float32)
    for g in range(NG):
        m, par = g // 2, g % 2
        cws[:, 128 * m + 64 * par: 128 * m + 64 * par + 48] = \
            cw_c[:, 48 * g: 48 * g + 48]
        for t_loc in range(TPG):
            t = t0 + TPG * g + t_loc
            rows = slice(64 * par + 6 * t_loc, 64 * par + 6 * t_loc + 6)
            tbv[rows, m] = b_all[t]
    return dict(cws=cws, tbv=tbv)


def _host_prep_core(c, xto, feat_attention, a0_all, b_all):
    t0 = T_C * c
    fa_c = feat_attention[:, D * t0: D * (t0 + T_C)]
    fap = np.zeros((F, NG * MROW), np.float32)
    ta0 = np.zeros((128, 4), np.float32)
    tbb = np.full((128, 4), 0.5, np.float32)
    for g in range(NG):
        fap[:, MROW * g: MROW * g + 48] = fa_c[:, 48 * g: 48 * g + 48]
        m, half = g // 2, g % 2
        for t_loc in range(TPG):
            t = t0 + TPG * g + t_loc
            rows = slice(64 * half + 6 * t_loc, 64 * half + 6 * t_loc + 6)
            ta0[rows, m] = a0_all[t]
            tbb[rows, m] = b_all[t]
    return dict(xt=xto, fap=fap, ta0=ta0, tbb=tbb)


def _enable_ldw_opt():
    """Turn on walrus's LDWEIGHTS dedup for this process's compiles
    (validated: identical results, fewer redundant weight loads)."""
    import concourse.bass_utils as bu
    if getattr(bu.run_command, "_ldw_opt", False):
        return
    orig = bu.run_command

    def patched(argv, **kw):
        argv = [a.replace("--enable-ldw-opt=false", "--enable-ldw-opt=true")
                for a in argv]
        return orig(argv, **kw)

    patched._ldw_opt = True
    bu.run_command = patched


def kernel(x, feat_attention, thresholds, log_temperatures, response, path_map):
    _enable_ldw_opt()
    x = np.ascontiguousarray(np.asarray(x, dtype=np.float32))
    feat_attention = np.asarray(feat_attention, dtype=np.float32)
    thresholds = np.asarray(thresholds, dtype=np.float32)
    log_temperatures = np.asarray(log_temperatures, dtype=np.float32)
    response = np.asarray(response, dtype=np.float32)

    fast = _is_oblivious(path_map)
    key = "fast" if fast else "generic"
    if key not in _CACHE:
        _CACHE[key] = (_build_program_fast() if fast
                       else _build_program_generic())
    nc = _CACHE[key]

    in_maps = []
    if fast:
        xto, cw_s, b_all = _host_prep_fast(x, feat_attention, thresholds,
                                           log_temperatures)
        for c in range(NCORES):
            m = _host_prep_fast_core(c, cw_s, b_all)
            m["xt"] = xto
            m["sel2e"] = _CACHE.setdefault("sel2e", _build_sel2_v2(0))
            m["sel2o"] = _CACHE.setdefault("sel2o", _build_sel2_v2(1))
            m["sel1e"] = _CACHE.setdefault("sel1e", _build_sel1_v2(0))
            m["sel1o"] = _CACHE.setdefault("sel1o", _build_sel1_v2(1))
            m["selh"] = _CACHE.setdefault("selh", _build_selh())
            m["resp2"] = _build_resp2(response[T_C * c:T_C * (c + 1)])
            in_maps.append(m)
    else:
        xto = np.ascontiguousarray(
            np.concatenate([x.T, np.ones((F, 2), np.float32)], axis=1))
        elt = np.exp(-log_temperatures)
        a0_all = 0.5 * elt
        b_all = 0.5 - 0.5 * thresholds * elt
        for c in range(NCORES):
            m = _host_prep_core(c, xto, feat_attention, a0_all, b_all)
            t0 = T_C * c
            if "selg" not in _CACHE:
                _CACHE["selg"] = _build_sel_generic(path_map)
            m["selz"] = _CACHE["selg"]
            m["rbd"] = _build_rbd_generic(response[t0:t0 + T_C])
            in_maps.append(m)

    _CACHE["in_maps"] = in_maps
    res = run_bass_kernel_spmd(nc, in_maps, core_ids=list(range(NCORES)))
    outs = [res.results[c]["out"].T for c in range(NCORES)]
    return np.ascontiguousarray(np.concatenate(outs, axis=1))


# revision 16
# speedup vs baseline: 1.0325x; 1.0199x over previous
"""Trainium2 Bass kernel for nn_DeTree (NODE-style oblivious decision ensemble).

Tree-sharded over 8 cores (64 trees/core), full batch per core.

Fast path (oblivious path_map), v2 — fully software-pipelined:
  Host folds softmax(feat_attention), the temperature scale (0.5*exp(-lt))
  and the softmax denominator into the fv weights `cws`, so the device
  pipeline is pure matmul/elementwise/act with no frontend exp:
    1. PE: fv(m,nh) = cws[:,m-block]^T @ x^T[, nh-half]   (f32r)
    2. DVE/GPSIMD: u = min(fv+b, 1); bins = max(u, EPS);
       omb = clamp(1-u, EPS, 1-EPS)  (pg tile, parity-swapped row layout)
    3. ACT: glog = ln(pg)
    4. PE: lo-sums S2 (16 combos/tree) + replicated hi-sums S1
       (3r x 4 combos/tree) via constant 0/1 selection matmuls.
    5. ACT: E2 = exp(S2), E1 = exp(S1)
    6. PE: M1[t,(r,hi)] = sum_lo resp[t,hi*16+lo,r] * E2[t,lo] (block-diag)
    7. DVE: P = M1 * E1
    8. PE: out[t*3+r] = sum_hi P, 4 groups accumulated per psum tile.
  All stages interleave per tree-group g (PE order: S(g), M1(g-1), OP(g-2))
  so PE/ACT/DVE/GPSIMD stay busy concurrently; input DMAs are spread
  across engine queues so the first fv matmul starts ~1us in.
Generic path (any path_map): 2-trees-per-matmul leaf log-sum (64 leaves),
exp, response block-diag accumulation (v1, unchanged).
"""
import numpy as np
from contextlib import ExitStack

import concourse.bass as bass
import concourse.bacc as bacc
import concourse.tile as tile
import concourse.mybir as mybir
from concourse.bass_utils import run_bass_kernel_spmd

F32 = mybir.dt.float32
F32R = mybir.dt.float32r
AF = mybir.ActivationFunctionType
ALU = mybir.AluOpType

B = 1024          # batch
F = 512           # in_features
T = 512           # num_trees
D = 6             # depth
R = 3             # response_dim
NLEAF = 64
NCORES = 8
T_C = T // NCORES          # 64 trees per core
TPG = 8                    # trees per gate-tile group
NG = T_C // TPG            # 8 groups per core
MROW = 64                  # padded rows per fv M-tile half (48 real + 16 pad)
NPAIR = T_C // 2           # generic path: 32 tree-pairs per core
PAIRS_PER_EG = 16
EPS = 2.0 ** -20
NH = 2                     # batch halves (1024 = 2 x 512)
BH = B // NH               # 512
NLO = 16                   # 2^4 lo-combos (depths 0..3)
NHI = 4                    # 2^2 hi-combos (depths 4..5)

_CACHE = {}


def _is_oblivious(path_map):
    pm = np.asarray(path_map).reshape(NLEAF, D)
    exp = np.array([[2 * j + ((l >> j) & 1) for j in range(D)]
                    for l in range(NLEAF)], dtype=pm.dtype)
    return bool(np.array_equal(pm, exp))


# ───────────────────────── fast (v2) constants ────────────────────────────
# pg row layout per group parity (within its [128, B] tile):
#   even g: bins rows 0..47,  omb rows 64..111  (src fv partitions 0..47)
#   odd  g: bins rows 64..111, omb rows 0..47   (src fv partitions 64..111)
# The bins write is partition-UNSHIFTED from fv and carries the per-row
# b-vector AP; the omb write derives from bins with constant scalars only.
# All partition starts are 0/64 (hardware requires starts in {0,32,64,96});
# rows 48..63 are never written (memset once per ring buffer to stay
# ln-safe), rows 112..127 are never read.

def _rowv2(parity, s, t_loc, d):
    """pg-tile row of gate (d, s) for local tree t_loc. s=0: bins, s=1: omb."""
    if parity == 0:
        base = 0 if s == 0 else 64
    else:
        base = 64 if s == 0 else 0
    return base + 6 * t_loc + d


def _build_sel2_v2(parity):
    """[128, 128] lo-sum selection: col = 16*t_loc + lo, depths 0..3."""
    S = np.zeros((128, 128), np.float32)
    for t_loc in range(TPG):
        for lo in range(NLO):
            col = NLO * t_loc + lo
            for j in range(4):
                S[_rowv2(parity, (lo >> j) & 1, t_loc, j), col] = 1.0
    return S


def _build_sel1_v2(parity):
    """[128, 96] replicated hi-sum selection: col = 12*t_loc + 4*r + hi."""
    S = np.zeros((128, 96), np.float32)
    for t_loc in range(TPG):
        for r in range(R):
            for hi in range(NHI):
                col = 12 * t_loc + 4 * r + hi
                for j in range(4, 6):
                    S[_rowv2(parity, (hi >> (j - 4)) & 1, t_loc, j), col] = 1.0
    return S


def _build_selh():
    """[96, 4*96] hi-reduce: 4 variants (group slot in psum accumulation).

    variant v: rows = P rows (12*t_loc + 4*r + hi), col = 24*v + 3*t_loc + r.
    """
    S = np.zeros((96, 4 * 96), np.float32)
    for v in range(4):
        for t_loc in range(TPG):
            for r in range(R):
                for hi in range(NHI):
                    S[12 * t_loc + 4 * r + hi, 96 * v + 24 * v + 3 * t_loc + r] = 1.0
    return S


def _build_resp2(response_core):
    """[128, NG*96]: per group g, rows 16*t_loc+lo, col 12*t_loc+4*r+hi =
    response[8g+t_loc, hi*16+lo, r]."""
    out = np.zeros((128, NG * 96), np.float32)
    for g in range(NG):
        for t_loc in range(TPG):
            t = TPG * g + t_loc
            for hi in range(NHI):
                for r in range(R):
                    out[NLO * t_loc:NLO * t_loc + NLO,
                        96 * g + 12 * t_loc + 4 * r + hi] = \
                        response_core[t, hi * NLO:(hi + 1) * NLO, r]
    return out


# ───────────────────────── generic-path constants (v1) ────────────────────

def _gate_row(t_loc, g):
    """v1 pg-tile row of gate g (= 2d+s) for local tree t_loc."""
    d, s = g // 2, g % 2
    return (64 if s else 0) + 6 * t_loc + d


def _build_sel_generic(path_map):
    pm = np.asarray(path_map).reshape(NLEAF, D)
    sel = np.zeros((4, 128, 128), np.float32)
    for k in range(4):
        for t01 in range(2):
            t_loc = 2 * k + t01
            for leaf in range(NLEAF):
                col = 64 * t01 + leaf
                for j in range(D):
                    sel[k, _gate_row(t_loc, int(pm[leaf, j])), col] += 1.0
    return np.ascontiguousarray(sel.transpose(1, 0, 2).reshape(128, 512))


def _build_rbd_generic(response_core):
    rbd = np.zeros((128, NPAIR * 96), np.float32)
    for p in range(NPAIR):
        q = p % PAIRS_PER_EG
        for t01 in range(2):
            t = 2 * p + t01
            c0 = 96 * p + 6 * q + 3 * t01
            rbd[64 * t01:64 * t01 + 64, c0:c0 + 3] = response_core[t]
    return rbd


# ───────────────────────── program builders ──────────────────────────────

def _patched_act_tables():
    """Force Ln+Exp onto the shared natural_log_exp_and_others table set
    so the ACT LUT isn't reloaded between ln and exp phases."""
    import concourse.bacc as bacc_mod
    from concourse.hw_specs import get_activation_tables as orig

    def patched(arch):
        tabs = orig(arch)
        if "natural_log_exp_and_others" in tabs:
            for name, funcs in tabs.items():
                if name != "natural_log_exp_and_others":
                    funcs.discard(AF.Ln)
                    funcs.discard(AF.Exp)
        return tabs

    class _Ctx:
        def __enter__(self):
            self.saved = bacc_mod.get_activation_tables
            bacc_mod.get_activation_tables = patched

        def __exit__(self, *a):
            bacc_mod.get_activation_tables = self.saved

    return _Ctx()


def _build_program_fast():
    nc = bacc.Bacc("TRN2", target_bir_lowering=False, debug=False,
                   num_devices=NCORES)
    xt = nc.dram_tensor("xt", [F, B], F32R, kind="ExternalInput")
    cws = nc.dram_tensor("cws", [F, NG * MROW], F32R, kind="ExternalInput")
    tbv = nc.dram_tensor("tbv", [128, 4], F32, kind="ExternalInput")
    sel2e = nc.dram_tensor("sel2e", [128, 128], F32R, kind="ExternalInput")
    sel2o = nc.dram_tensor("sel2o", [128, 128], F32R, kind="ExternalInput")
    sel1e = nc.dram_tensor("sel1e", [128, 96], F32R, kind="ExternalInput")
    sel1o = nc.dram_tensor("sel1o", [128, 96], F32R, kind="ExternalInput")
    selh = nc.dram_tensor("selh", [96, 4 * 96], F32R, kind="ExternalInput")
    resp2 = nc.dram_tensor("resp2", [128, NG * 96], F32R, kind="ExternalInput")
    out = nc.dram_tensor("out", [T_C * R, B], F32, kind="ExternalOutput")

    with tile.TileContext(nc) as tc, ExitStack() as ctx:
        cpool = ctx.enter_context(tc.tile_pool(name="consts", bufs=1))
        txt = [cpool.tile([128, B], F32R, name=f"txt{k}", tag=f"xt{k}")
               for k in range(4)]
        tcw = [cpool.tile([128, NG * MROW], F32R, name=f"tcw{k}", tag=f"cw{k}")
               for k in range(4)]
        tb = cpool.tile([128, 4], F32)
        tsel2 = [cpool.tile([128, 128], F32R, name=f"tsel2{p}", tag=f"s2{p}")
                 for p in range(2)]
        tsel1 = [cpool.tile([128, 96], F32R, name=f"tsel1{p}", tag=f"s1{p}")
                 for p in range(2)]
        tselh = cpool.tile([96, 4 * 96], F32R)
        tresp2 = cpool.tile([128, NG * 96], F32R)

        # Input DMAs spread across the three DMA-capable queues
        # (sync/scalar/gpsimd); the pair (cw[k], xt[k]) needed first lands
        # first. Tensor queue stays DMA-free.
        nc.sync.dma_start(tcw[0][:], cws[0:128, :])
        nc.scalar.dma_start(txt[0][:], xt[0:128, :])
        nc.gpsimd.dma_start(tcw[3][:], cws[384:512, :])
        nc.sync.dma_start(txt[1][:], xt[128:256, :])
        nc.scalar.dma_start(tcw[1][:], cws[128:256, :])
        nc.gpsimd.dma_start(txt[2][:], xt[256:384, :])
        nc.sync.dma_start(tcw[2][:], cws[256:384, :])
        nc.scalar.dma_start(tb[:], tbv[:])
        nc.gpsimd.dma_start(tsel2[1][:], sel2o[:])
        nc.sync.dma_start(txt[3][:], xt[384:512, :])
        nc.scalar.dma_start(tresp2[:], resp2[:])
        nc.gpsimd.dma_start(tsel1[1][:], sel1o[:])
        nc.sync.dma_start(tsel2[0][:], sel2e[:])
        nc.gpsimd.dma_start(tselh[:], selh[:])
        nc.sync.dma_start(tsel1[0][:], sel1e[:])

        fvpool = ctx.enter_context(tc.tile_pool(name="fvp", bufs=2,
                                                space="PSUM"))
        spool = ctx.enter_context(tc.tile_pool(name="sp", bufs=4,
                                               space="PSUM"))
        opool = ctx.enter_context(tc.tile_pool(name="opp", bufs=2,
                                               space="PSUM"))
        pgpool = ctx.enter_context(tc.tile_pool(name="pgp", bufs=3))
        glpool = ctx.enter_context(tc.tile_pool(name="glp", bufs=3))
        e2pool = ctx.enter_context(tc.tile_pool(name="e2p", bufs=3))
        e1pool = ctx.enter_context(tc.tile_pool(name="e1p", bufs=3))
        pppool = ctx.enter_context(tc.tile_pool(name="ppp", bufs=3))
        evpool = ctx.enter_context(tc.tile_pool(name="evp", bufs=4))

        fv = {}        # (m, nh) -> psum tile
        pg = [None] * NG
        glog = [None] * NG
        e2 = [None] * NG
        e1 = [None] * NG
        pp = [None] * NG
        m1 = {}        # (g, nh)
        op = {}        # (eg, nh)

        def emit_fv(m):
            for nh in range(NH):
                fv[(m, nh)] = fvpool.tile([128, BH], F32, name=f"fv{m}_{nh}",
                                          tag="fv")
            for k in range(4):
                for nh in range(NH):
                    nc.tensor.matmul(fv[(m, nh)][:],
                                     tcw[k][:, 128 * m:128 * (m + 1)],
                                     txt[k][:, BH * nh:BH * (nh + 1)],
                                     start=(k == 0), stop=(k == 3))

        def emit_eltw_ln(g):
            m, par = g // 2, g % 2
            rb = 64 * par                       # fv source row base
            bins = _rowv2(par, 0, 0, 0)         # 0 or 64
            omb = _rowv2(par, 1, 0, 0)          # 64 or 0
            # 64-row ops (48 real + 16 fv-pad rows, which give finite,
            # zero-weighted gate values) so every pg row is written each
            # iteration and all partition starts are 0/64.
            t = pgpool.tile([128, B], F32, name=f"pg{g}", tag="pg")
            pg[g] = t
            gl = glpool.tile([128, B], F32R, name=f"gl{g}", tag="gl")
            glog[g] = gl
            for nh in range(NH):
                hc = slice(BH * nh, BH * (nh + 1))
                src = fv[(m, nh)][rb:rb + 64, :]
                # u = min(fv + b, 1)  (unshifted; per-row b AP)
                nc.vector.tensor_scalar(t[bins:bins + 64, hc], src,
                                        tb[rb:rb + 64, m:m + 1], 1.0,
                                        ALU.add, ALU.min)
                # omb = clamp(1 - u, EPS, 1-EPS)   (shifted, consts only;
                # in-place ops stay on DVE — in-place GPSIMD is ~12x slower)
                nc.gpsimd.tensor_scalar(t[omb:omb + 64, hc],
                                        t[bins:bins + 64, hc],
                                        -1.0, 1.0, ALU.mult, ALU.add)
                nc.vector.tensor_scalar(t[omb:omb + 64, hc],
                                        t[omb:omb + 64, hc],
                                        1.0 - EPS, EPS, ALU.min, ALU.max)
                # bins = max(u, EPS)
                nc.vector.tensor_scalar(t[bins:bins + 64, hc],
                                        t[bins:bins + 64, hc],
                                        EPS, EPS, ALU.max, ALU.max)
                nc.scalar.activation(gl[:, hc], t[:, hc], AF.Ln)

        def emit_s(g):
            par = g % 2
            s2 = [spool.tile([128, BH], F32, name=f"s2_{g}_{nh}", tag="s")
                  for nh in range(NH)]
            s1 = [spool.tile([128, BH], F32, name=f"s1_{g}_{nh}", tag="s")
                  for nh in range(NH)]
            for nh in range(NH):
                nc.tensor.matmul(s2[nh][:], tsel2[par][:],
                                 glog[g][:, BH * nh:BH * (nh + 1)],
                                 start=True, stop=True)
            for nh in range(NH):
                nc.tensor.matmul(s1[nh][0:96, :], tsel1[par][:],
                                 glog[g][:, BH * nh:BH * (nh + 1)],
                                 start=True, stop=True)
            ee2 = e2pool.tile([128, B], F32R, name=f"e2_{g}", tag="e2")
            e2[g] = ee2
            ee1 = e1pool.tile([96, B], F32R, name=f"e1_{g}", tag="e1")
            e1[g] = ee1
            for nh in range(NH):
                nc.scalar.activation(ee2[:, BH * nh:BH * (nh + 1)],
                                     s2[nh][:], AF.Exp)
            for nh in range(NH):
                nc.scalar.activation(ee1[:, BH * nh:BH * (nh + 1)],
                                     s1[nh][0:96, :], AF.Exp)

        def emit_m1(g):
            t = pppool.tile([96, B], F32R, name=f"pp{g}", tag="pp")
            pp[g] = t
            for nh in range(NH):
                m1[(g, nh)] = spool.tile([128, BH], F32, name=f"m1_{g}_{nh}",
                                         tag="s")
                nc.tensor.matmul(m1[(g, nh)][0:96, :],
                                 tresp2[:, 96 * g:96 * (g + 1)],
                                 e2[g][:, BH * nh:BH * (nh + 1)],
                                 start=True, stop=True)
            for nh in range(NH):
                nc.vector.tensor_tensor(t[:, BH * nh:BH * (nh + 1)],
                                        m1[(g, nh)][0:96, :],
                                        e1[g][:, BH * nh:BH * (nh + 1)],
                                        ALU.mult)

        def emit_op(g):
            eg, v = g // 4, g % 4
            if v == 0:
                for nh in range(NH):
                    op[(eg, nh)] = opool.tile([96, BH], F32,
                                              name=f"op{eg}_{nh}", tag="op")
            for nh in range(NH):
                nc.tensor.matmul(op[(eg, nh)][:],
                                 tselh[:, 96 * v:96 * (v + 1)],
                                 pp[g][:, BH * nh:BH * (nh + 1)],
                                 start=(v == 0), stop=(v == 3),
                                 skip_group_check=True)
            if v == 3:
                for nh in range(NH):
                    ev = evpool.tile([96, BH], F32, name=f"ev{eg}_{nh}",
                                     tag="ev")
                    nc.vector.tensor_copy(ev[:], op[(eg, nh)][:])
                    nc.sync.dma_start(
                        out[96 * eg:96 * (eg + 1), BH * nh:BH * (nh + 1)],
                        ev[:])

        # ── pipeline ──
        emit_fv(0)
        emit_fv(1)
        for g in range(NG):
            emit_eltw_ln(g)
            emit_s(g)
            if g == 0 and NG > 2:
                emit_fv(2)
            if g == 1 and NG > 3:
                emit_fv(3)
            if g >= 1:
                emit_m1(g - 1)
            if g >= 2:
                emit_op(g - 2)
        emit_m1(NG - 1)
        emit_op(NG - 2)
        emit_op(NG - 1)

    with _patched_act_tables():
        nc.compile()
    return nc


# ───────────────────────── generic path (v1, unchanged) ──────────────────

def _common_frontend(nc, tc, ctx):
    """DMA inputs and ecw = exp(feat_attention) tiles."""
    xt = nc.dram_tensor("xt", [F, B + 2], F32R, kind="ExternalInput")
    fap = nc.dram_tensor("fap", [F, NG * MROW], F32, kind="ExternalInput")
    ta0 = nc.dram_tensor("ta0", [128, 4], F32, kind="ExternalInput")
    tbb = nc.dram_tensor("tbb", [128, 4], F32, kind="ExternalInput")

    cpool = ctx.enter_context(tc.tile_pool(name="consts", bufs=1))
    txt = [cpool.tile([128, B + 2], F32R, name=f"txt{k}", tag=f"xt{k}")
           for k in range(4)]
    tfap = [cpool.tile([128, NG * MROW], F32, name=f"tfap{k}", tag=f"fap{k}")
            for k in range(4)]
    tecw = [cpool.tile([128, NG * MROW], F32R, name=f"tecw{k}", tag=f"ecw{k}")
            for k in range(4)]
    tta0 = cpool.tile([128, 4], F32)
    ttb = cpool.tile([128, 4], F32)
    tra = cpool.tile([128, 4], F32)
    trz = cpool.tile([128, 8], F32)

    for k in range(4):
        nc.sync.dma_start(txt[k][:], xt[128 * k:128 * k + 128, :])
        nc.sync.dma_start(tfap[k][:], fap[128 * k:128 * k + 128, :])
    nc.sync.dma_start(tta0[:], ta0[:])
    nc.sync.dma_start(ttb[:], tbb[:])

    for k in range(4):
        nc.scalar.activation(tecw[k][:], tfap[k][:], AF.Exp)

    return cpool, txt, tecw, tra, trz, tta0, ttb


def _emit_glogs(nc, tc, ctx, txt, tecw, tra, trz, tta0, ttb, glog_bufs):
    """Per-group gate-log tiles via M=128 fv matmuls with fused Z columns.
    Returns list of glog APs."""
    glpool = ctx.enter_context(tc.tile_pool(name="glp", bufs=glog_bufs))
    lctx = ctx.enter_context(ExitStack())
    fvpool = lctx.enter_context(tc.tile_pool(name="fvps", bufs=1, space="PSUM"))
    wpool = lctx.enter_context(tc.tile_pool(name="work", bufs=2))
    pgpool = lctx.enter_context(tc.tile_pool(name="pgp", bufs=2))
    glogs = [None] * NG
    for m in range(4):          # M-tile = 2 gate groups (2m, 2m+1)
        fv = fvpool.tile([128, B + 2], F32, name=f"fv{m}", tag="fv")
        for k in range(4):
            for off, n in ((0, 512), (512, 512), (1024, 2)):
                nc.tensor.matmul(fv[:, off:off + n],
                                 tecw[k][:, 128 * m:128 * (m + 1)],
                                 txt[k][:, off:off + n],
                                 start=(k == 0), stop=(k == 3))
        nc.vector.reciprocal(trz[:, 2 * m:2 * m + 2], fv[:, 1024:1026])
        nc.vector.tensor_mul(tra[:, m:m + 1], tta0[:, m:m + 1],
                             trz[:, 2 * m:2 * m + 1])
        tmp = wpool.tile([128, B], F32, name=f"tmp{m}", tag="tmp")
        nc.vector.tensor_scalar(tmp[:], fv[:, 0:1024], tra[:, m:m + 1],
                                ttb[:, m:m + 1], ALU.mult, ALU.add)
        for half in range(2):
            g = 2 * m + half
            th = tmp[64 * half:64 * half + 64, :]
            pgt = pgpool.tile([128, B], F32R, name=f"pg{g}", tag="pg")
            nc.gpsimd.tensor_scalar(pgt[0:64, :], th, 1.0, EPS, ALU.min, ALU.max)
            nc.gpsimd.tensor_scalar(pgt[64:128, :], th, -1.0, 1.0,
                                    ALU.mult, ALU.add)
            nc.vector.tensor_scalar(pgt[64:128, :], pgt[64:128, :], 1.0 - EPS,
                                    EPS, ALU.min, ALU.max)
            gl = glpool.tile([128, B], F32R, name=f"glog{g}", tag="glog")
            nc.scalar.activation(gl[:], pgt[:], AF.Ln)
            glogs[g] = gl
    lctx.close()
    return glogs


def _build_program_generic():
    nc = bacc.Bacc("TRN2", target_bir_lowering=False, debug=False,
                   num_devices=NCORES)
    selz = nc.dram_tensor("selz", [128, 512], F32R, kind="ExternalInput")
    rbd = nc.dram_tensor("rbd", [128, NPAIR * 96], F32R, kind="ExternalInput")
    out = nc.dram_tensor("out", [T_C * R, B], F32, kind="ExternalOutput")

    with tile.TileContext(nc) as tc, ExitStack() as ctx:
        cpool, txt, tecw, tra, trz, tta0, ttb = _common_frontend(nc, tc, ctx)
        tselz = cpool.tile([128, 512], F32R)
        trbd = cpool.tile([128, NPAIR * 96], F32R)
        nc.sync.dma_start(tselz[:], selz[:])
        nc.sync.dma_start(trbd[:], rbd[:])

        glogs = _emit_glogs(nc, tc, ctx, txt, tecw, tra, trz, tta0, ttb,
                            glog_bufs=3)

        rwpool = ctx.enter_context(tc.tile_pool(name="rwp", bufs=3))
        evpool = ctx.enter_context(tc.tile_pool(name="evp", bufs=2))
        with (
            tc.tile_pool(name="sps", bufs=2, space="PSUM") as spool,
            tc.tile_pool(name="ops", bufs=1, space="PSUM") as opool,
        ):
            for eg in range(2):
                opt = opool.tile([96, B], F32, name=f"op{eg}", tag="outp")
                for gi in range(NG // 2):
                    g = eg * (NG // 2) + gi
                    for k in range(4):
                        p = 4 * g + k
                        q = p % PAIRS_PER_EG
                        sp = spool.tile([128, B], F32, name=f"sp{p}", tag="s")
                        for nh in range(NH):
                            nc.tensor.matmul(sp[:, 512 * nh:512 * (nh + 1)],
                                             tselz[:, 128 * k:128 * (k + 1)],
                                             glogs[g][:, 512 * nh:512 * (nh + 1)],
                                             start=True, stop=True)
                        rw = rwpool.tile([128, B], F32R, name=f"rw{p}", tag="rw")
                        nc.scalar.activation(rw[:], sp[:], AF.Exp)
                        for nh in range(NH):
                            nc.tensor.matmul(opt[:, 512 * nh:512 * (nh + 1)],
                                             trbd[:, 96 * p:96 * (p + 1)],
                                             rw[:, 512 * nh:512 * (nh + 1)],
                                             start=(q == 0),
                                             stop=(q == PAIRS_PER_EG - 1),
                                             skip_group_check=True)
                ev = evpool.tile([96, B], F32, name=f"ev{eg}", tag="ev")
                nc.vector.tensor_copy(ev[:], opt[:])
                nc.sync.dma_start(out[96 * eg:96 * (eg + 1), :], ev[:])

    with _patched_act_tables():
        nc.compile()
    return nc


# ───────────────────────── host entry point ──────────────────────────────

def _host_prep_fast(x, feat_attention, thresholds, log_temperatures):
    """Shared (core-independent) fast-path prep: xt, folded weights, b."""
    xto = np.ascontiguousarray(x.T)
    ecw = np.exp(feat_attention)
    cw = ecw / ecw.sum(axis=0, keepdims=True)          # softmax over features
    elt = np.exp(-log_temperatures)                    # [T, D]
    a0 = 0.5 * elt
    cw_s = cw.reshape(F, T, D) * a0[None]              # fold scale into weights
    cw_s = cw_s.reshape(F, T * D)
    b_all = 0.5 - 0.5 * thresholds * elt               # [T, D]
    return xto, cw_s, b_all


def _host_prep_fast_core(c, cw_s, b_all):
    t0 = T_C * c
    cw_c = cw_s[:, D * t0: D * (t0 + T_C)]             # [F, 384]
    cws = np.zeros((F, NG * MROW), np.float32)
    tbv = np.zeros((128, 4), np.float32)
    for g in range(NG):
        m, par = g // 2, g % 2
        cws[:, 128 * m + 64 * par: 128 * m + 64 * par + 48] = \
            cw_c[:, 48 * g: 48 * g + 48]
        for t_loc in range(TPG):
            t = t0 + TPG * g + t_loc
            rows = slice(64 * par + 6 * t_loc, 64 * par + 6 * t_loc + 6)
            tbv[rows, m] = b_all[t]
    return dict(cws=cws, tbv=tbv)


def _host_prep_core(c, xto, feat_attention, a0_all, b_all):
    t0 = T_C * c
    fa_c = feat_attention[:, D * t0: D * (t0 + T_C)]
    fap = np.zeros((F, NG * MROW), np.float32)
    ta0 = np.zeros((128, 4), np.float32)
    tbb = np.full((128, 4), 0.5, np.float32)
    for g in range(NG):
        fap[:, MROW * g: MROW * g + 48] = fa_c[:, 48 * g: 48 * g + 48]
        m, half = g // 2, g % 2
        for t_loc in range(TPG):
            t = t0 + TPG * g + t_loc
            rows = slice(64 * half + 6 * t_loc, 64 * half + 6 * t_loc + 6)
            ta0[rows, m] = a0_all[t]
            tbb[rows, m] = b_all[t]
    return dict(xt=xto, fap=fap, ta0=ta0, tbb=tbb)


def _enable_ldw_opt():
    """Turn on walrus's LDWEIGHTS dedup for this process's compiles
    (validated: identical results, fewer redundant weight loads)."""
    import concourse.bass_utils as bu
    if getattr(bu.run_command, "_ldw_opt", False):
        return
    orig = bu.run_command

    def patched(argv, **kw):
        argv = [a.replace("--enable-ldw-opt=false", "--enable-ldw-opt=true")
                for a in argv]
        return orig(argv, **kw)

    patched._ldw_opt = True
    bu.run_command = patched


def kernel(x, feat_attention, thresholds, log_temperatures, response, path_map):
    _enable_ldw_opt()
    x = np.ascontiguousarray(np.asarray(x, dtype=np.float32))
    feat_attention = np.asarray(feat_attention, dtype=np.float32)
    thresholds = np.asarray(thresholds, dtype=np.float32)
    log_temperatures = np.asarray(log_temperatures, dtype=np.float32)
    response = np.asarray(response, dtype=np.float32)

    fast = _is_oblivious(path_map)
    key = "fast" if fast else "generic"
    if key not in _CACHE:
        _CACHE[key] = (_build_program_fast() if fast
                       else _build_program_generic())
    nc = _CACHE[key]

    in_maps = []
    if fast:
        xto, cw_s, b_all = _host_prep_fast(x, feat_attention, thresholds,
                                           log_temperatures)
        for c in range(NCORES):
            m = _host_prep_fast_core(c, cw_s, b_all)
            m["xt"] = xto
            m["sel2e"] = _CACHE.setdefault("sel2e", _build_sel2_v2(0))
            m["sel2o"] = _CACHE.setdefault("sel2o", _build_sel2_v2(1))
            m["sel1e"] = _CACHE.setdefault("sel1e", _build_sel1_v2(0))
            m["sel1o"] = _CACHE.setdefault("sel1o", _build_sel1_v2(1))
            m["selh"] = _CACHE.setdefault("selh", _build_selh())
            m["resp2"] = _build_resp2(response[T_C * c:T_C * (c + 1)])
            in_maps.append(m)
    else:
        xto = np.ascontiguousarray(
            np.concatenate([x.T, np.ones((F, 2), np.float32)], axis=1))
        elt = np.exp(-log_temperatures)
        a0_all = 0.5 * elt
        b_all = 0.5 - 0.5 * thresholds * elt
        for c in range(NCORES):
            m = _host_prep_core(c, xto, feat_attention, a0_all, b_all)
            t0 = T_C * c
            if "selg" not in _CACHE:
                _CACHE["selg"] = _build_sel_generic(path_map)
            m["selz"] = _CACHE["selg"]
            m["rbd"] = _build_rbd_generic(response[t0:t0 + T_C])
            in_maps.append(m)

    _CACHE["in_maps"] = in_maps
    res = run_bass_kernel_spmd(nc, in_maps, core_ids=list(range(NCORES)))
    outs = [res.results[c]["out"].T for c in range(NCORES)]
    return np.ascontiguousarray(np.concatenate(outs, axis=1))
